# revision 3
# baseline (speedup 1.0000x reference)
"""Trainium2 Bass kernel for nn_Net_74259984548321 — transposed-recurrence design.

v4 design (data-parallel over batch, 8 rows/core):
  - ALL recurrence state transposed: [128 part = gate/hidden dim, 8 = batch].
    Per-op cost is free-size-driven, so elementwise ops are ~10x cheaper than
    the row-major baseline.
  - Sigmoid-free LSTM cell: sigma(x) = (1+tanh(x/2))/2 with the 1/2 scales
    folded into host-preprocessed weights (i/f/o gate rows x0.5) and the
    h-convention H=2h (all h-consuming weight cols x0.5). One tanh covers all
    8 gate chunks -> 2 ACT ops per cell, all Tanh. Exp (attention softmax, CE)
    shares act-table set 0 with Tanh -> zero table swaps until the final Ln.
  - Gates accumulate in [128, 8, 8] PSUM via tiny N=8 matmuls: identity-inject
    of precomputed input projections + fp8 DoubleRow h-recurrence.
  - P1 (feat @ e1_Wih) computed transposed, interleaved with the encoder.
  - Decoder: d1 chain runs ahead (no input dependence); attention context is
    folded into the d2 gate matmul via M2R[t,g,b] = Wd2hh @ h2seq (precomputed
    once) so no context vector is materialized. Softmax sum via gpsimd
    partition_all_reduce; normalization via reciprocal+mult.
  - CE: logits in 1024-col chunks, exp+accum on ACT; target logit via host
    argmax + gathered wo_tgt columns -> one fused mult+accum; host adds the
    (input-only) target-bias sum.
"""

import os
import numpy as np

EBF16 = os.environ.get("EBF16", "0") == "1"

B, T, FEAT, H, V, L = 64, 80, 4096, 256, 8000, 32
DEC = L - 1
NCORES = 8
BS = B // NCORES          # 8
G = 4 * H                 # 1024
NCH = 8                   # gate chunks of 128
K2 = 16                   # feat-dim DoubleRow pairs (4096 = 16*2*128)
NRB = 5                   # P1 row blocks of 128 rows (= 16 steps)
TB = T * BS               # 640 encoder rows
CROWS = DEC * BS          # 248 decoder rows
NVC = 8                   # CE vocab chunks
VCW = [1024] * 7 + [832]  # chunk widths (sum = 8000)
DLEAD = 3                 # d1-chain lead over d2

_cache = {}


def _build_program():
    import concourse.tile as tile
    from concourse import bacc, mybir, bass_isa
    from concourse.bass import ts, ds
    from concourse.masks import make_identity
    from contextlib import ExitStack

    fp = mybir.dt.float32
    bf = mybir.dt.bfloat16
    f8 = mybir.dt.float8e4
    AF = mybir.ActivationFunctionType
    ALU = mybir.AluOpType
    AX = mybir.AxisListType
    MPM = mybir.MatmulPerfMode
    DR = MPM.DoubleRow

    nc = bacc.Bacc(None, target_bir_lowering=False)

    featT_d = nc.dram_tensor("featT", [128, NRB, K2, 2, 128], f8, kind="ExternalInput")
    w1T_d = nc.dram_tensor("w1T", [128, K2, 2, NCH, 128], f8, kind="ExternalInput")
    w1hT_d = nc.dram_tensor("w1hT", [128, 2, NCH, 128], f8, kind="ExternalInput")
    w2iT_d = nc.dram_tensor("w2iT", [128, 2, NCH, 128], f8, kind="ExternalInput")
    w2hT_d = nc.dram_tensor("w2hT", [128, 2, NCH, 128], f8, kind="ExternalInput")
    wd1T_d = nc.dram_tensor("wd1T", [128, 2, NCH, 128], f8, kind="ExternalInput")
    wd2iT_d = nc.dram_tensor("wd2iT", [128, 2, NCH, 128], f8, kind="ExternalInput")
    wd2hT_d = nc.dram_tensor("wd2hT", [128, 2, NCH, 128], f8, kind="ExternalInput")
    wd2cT_d = nc.dram_tensor("wd2cT", [128, 2, NCH, 128], bf, kind="ExternalInput")
    wh2rhs_d = nc.dram_tensor("wh2rhs", [128, 2, G], f8, kind="ExternalInput")
    woT_d = nc.dram_tensor("woT", [128, 2, V], f8, kind="ExternalInput")
    wotgt_d = nc.dram_tensor("wotgt", [128, 2, DEC, BS], f8, kind="ExternalInput")
    capT_d = nc.dram_tensor("capT", [128, 2, CROWS], bf, kind="ExternalInput")
    b1T_d = nc.dram_tensor("b1T", [128, NCH], fp, kind="ExternalInput")
    bd2T_d = nc.dram_tensor("bd2T", [128, NCH], fp, kind="ExternalInput")
    brows_d = nc.dram_tensor("brows", [1, 2 * G + V], bf, kind="ExternalInput")
    out_d = nc.dram_tensor("partial", [1, 1], fp, kind="ExternalOutput")

    with tile.TileContext(nc) as tc:
        with ExitStack() as ctx:
            const = ctx.enter_context(tc.tile_pool(name="const", bufs=1))
            wpool = ctx.enter_context(tc.tile_pool(name="w", bufs=1))
            state = ctx.enter_context(tc.tile_pool(name="state", bufs=1))
            acts = ctx.enter_context(tc.tile_pool(name="acts", bufs=8))
            gps = ctx.enter_context(tc.tile_pool(name="gps", bufs=3, space="PSUM"))

            # ---------- constants ----------
            identb = const.tile([128, 128], bf, tag="idb")
            make_identity(nc, identb)
            ones1 = const.tile([1, 128], bf, tag="ones1")
            nc.vector.memset(ones1, 1.0)
            onesc = const.tile([128, 1], fp, tag="onesc")
            nc.vector.memset(onesc, 1.0)

            # ---------- persistent weights / rows ----------
            brows = wpool.tile([1, 2 * G + V], bf, tag="brows")
            nc.sync.dma_start(brows, brows_d[:, :])
            b2row = brows[:, 0:G]
            bd1row = brows[:, G : 2 * G]
            boutrow = brows[:, 2 * G : 2 * G + V]
            b1T = wpool.tile([128, NCH], fp, tag="b1T")
            nc.sync.dma_start(b1T, b1T_d[:, :])
            bd2T = wpool.tile([128, NCH], fp, tag="bd2T")
            nc.sync.dma_start(bd2T, bd2T_d[:, :])

            w1hT = wpool.tile([128, 2, NCH, 128], f8, tag="w1hT")
            w2iT = wpool.tile([128, 2, NCH, 128], f8, tag="w2iT")
            w2hT = wpool.tile([128, 2, NCH, 128], f8, tag="w2hT")
            wd1T = wpool.tile([128, 2, NCH, 128], f8, tag="wd1T")
            wd2iT = wpool.tile([128, 2, NCH, 128], f8, tag="wd2iT")
            wd2hT = wpool.tile([128, 2, NCH, 128], f8, tag="wd2hT")
            wd2cT = wpool.tile([128, 2, NCH, 128], bf, tag="wd2cT")
            capT = wpool.tile([128, 2, CROWS], bf, tag="capT")
            wh2rhs = wpool.tile([128, 2, G], f8, tag="wh2rhs")
            wotgt = wpool.tile([128, 2, DEC, BS], f8, tag="wotgt")
            wo = wpool.tile([128, 2, V], f8, tag="wo")
            # DMAs for these are emitted after the critical feat/w1 uploads

            # ---------- persistent state ----------
            h1Tp = state.tile([128, 2, 16], f8, tag="h1T")
            nc.vector.memset(h1Tp, 0.0)
            h1T = h1Tp[:, :, 0:BS]
            h2seqT = state.tile([128, 2, T, BS], f8, tag="h2seqT")
            # t-dim padded to 32 so fp8 DoubleRow ko-stride (32*8) is 16B-aligned
            h1decT = state.tile([128, 2, 32, BS], f8, tag="h1decT")
            h2decT = state.tile([128, 2, 32, BS], f8, tag="h2decT")
            edt = bf if EBF16 else fp
            E1t = state.tile([128, 10, BS], edt, tag="E1")
            E2t = state.tile([128, 10, BS], edt, tag="E2")
            nc.vector.memset(E1t, 0.0)
            nc.vector.memset(E2t, 0.0)
            g1T = state.tile([128, NCH, TB], bf, tag="g1T")
            capg = state.tile([128, NCH, CROWS], bf, tag="capg")
            M2R = state.tile([80, BS, G], bf, tag="M2R")
            s_all = state.tile([128, 2, NVC], fp, tag="s_all")

            # ================= LSTM cell elementwise =================
            # E layout: 0:2=sO 2:4=sI 4:6=sF 6:8=tg 8:10=W(=2c)
            def cell(ps, E, hT_dst):
                nc.scalar.activation(E[:, 0:8, :], ps[:, :, :], AF.Tanh)
                uv = acts.tile([128, 4, BS], edt, tag="uv")
                nc.vector.scalar_tensor_tensor(
                    uv, E[:, 2:6, :], 1.0, E[:, 6:10, :],
                    op0=ALU.add, op1=ALU.mult,
                )
                nc.vector.scalar_tensor_tensor(
                    E[:, 8:10, :], uv[:, 2:4, :], 0.5, uv[:, 0:2, :],
                    op0=ALU.mult, op1=ALU.add,
                )
                th = acts.tile([128, 2, BS], edt, tag="th")
                nc.scalar.activation(th, E[:, 8:10, :], AF.Tanh, scale=0.5)
                nc.vector.scalar_tensor_tensor(
                    hT_dst, E[:, 0:2, :], 1.0, th, op0=ALU.add, op1=ALU.mult
                )

            # ================= encoder matmul emitters =================
            def mm1(t):
                ps = gps.tile([128, NCH, BS], fp, tag="cell", name="ps1",
                              padded_shape=[128, NCH, 64])
                last = t > 0
                for c in range(NCH):
                    nc.tensor.matmul(
                        ps[:, c, :], identb, g1T[:, c, ts(t, BS)],
                        start=True, stop=not last,
                    )
                if t > 0:
                    for c in range(NCH):
                        nc.tensor.matmul(
                            ps[:, c, :], w1hT[:, :, c, :], h1T,
                            start=False, stop=True, perf_mode=DR,
                        )
                return ps

            def mm2(t):
                ps = gps.tile([128, NCH, BS], fp, tag="cell", name="ps2",
                              padded_shape=[128, NCH, 64])
                for c in range(NCH):
                    nc.tensor.matmul(
                        ps[:, c, :], b2row[:, ts(c, 128)], ones1[:, 0:BS],
                        start=True, stop=False,
                    )
                if t > 0:
                    for c in range(NCH):
                        nc.tensor.matmul(
                            ps[:, c, :], w2hT[:, :, c, :], h2seqT[:, :, t - 1, :],
                            start=False, stop=False, perf_mode=DR,
                        )
                for c in range(NCH):
                    nc.tensor.matmul(
                        ps[:, c, :], w2iT[:, :, c, :], h1T,
                        start=False, stop=True, perf_mode=DR,
                    )
                return ps

            def emit_m2r(pool, t0, tn, tag):
                for b in range(BS):
                    psm = pool.tile([80, G], fp, tag=tag,
                                    padded_shape=[80, 1024])
                    for j in range(2):
                        nc.tensor.matmul(
                            psm[0:tn, ts(j, 512)], h2seqT[:, :, ds(t0, tn), b],
                            wh2rhs[:, :, ts(j, 512)],
                            start=True, stop=True, perf_mode=DR,
                        )
                    nc.scalar.copy(M2R[ds(t0, tn), b, 0:512], psm[0:tn, 0:512])
                    nc.vector.tensor_copy(
                        M2R[ds(t0, tn), b, 512:1024], psm[0:tn, 512:1024]
                    )

            # ================= phase 1: encoder + P1 =================
            with ExitStack() as p1ctx:
                ftp = p1ctx.enter_context(tc.tile_pool(name="ftp", bufs=1))
                p1ps = p1ctx.enter_context(
                    tc.tile_pool(name="p1ps", bufs=2, space="PSUM")
                )
                w1sb = ftp.tile([128, K2, 2, NCH, 128], f8, tag="w1")
                ftall = ftp.tile([128, NRB, K2, 2, 128], f8, tag="ft")
                nc.sync.dma_start(ftall[:, 0, :, :, :], featT_d[:, 0, :, :, :])
                for kh in range(4):
                    nc.sync.dma_start(
                        w1sb[:, ts(kh, K2 // 4), :, :, :],
                        w1T_d[:, ts(kh, K2 // 4), :, :, :],
                    )
                for rb in range(1, NRB):
                    nc.sync.dma_start(
                        ftall[:, rb, :, :, :], featT_d[:, rb, :, :, :]
                    )
                nc.sync.dma_start(w1hT, w1hT_d[:, :, :, :])
                nc.sync.dma_start(w2iT, w2iT_d[:, :, :, :])
                nc.sync.dma_start(w2hT, w2hT_d[:, :, :, :])
                nc.sync.dma_start(wd1T, wd1T_d[:, :, :, :])
                nc.sync.dma_start(wd2iT, wd2iT_d[:, :, :, :])
                nc.sync.dma_start(wd2hT, wd2hT_d[:, :, :, :])
                nc.sync.dma_start(wd2cT, wd2cT_d[:, :, :, :])
                nc.sync.dma_start(capT, capT_d[:, :, :])
                nc.sync.dma_start(wh2rhs, wh2rhs_d[:, :, :])
                nc.sync.dma_start(wotgt, wotgt_d[:, :, :, :])
                for hh in range(4):
                    nc.sync.dma_start(
                        wo[:, :, ts(hh, V // 4)], woT_d[:, :, ts(hh, V // 4)]
                    )

                def emit_p1_block(rb):
                    # k2-major so MMs pipeline with the w1 DMA chunks; all 8
                    # chunk-psums live in one 2-bank tile.
                    ps = p1ps.tile([128, NCH, 128], fp, tag="p1")
                    for k2 in range(K2):
                        for mc in range(NCH):
                            nc.tensor.matmul(
                                ps[:, mc, :], w1sb[:, k2, :, mc, :],
                                ftall[:, rb, k2, :, :],
                                start=(k2 == 0), stop=(k2 == K2 - 1),
                                perf_mode=DR,
                            )
                    for mc in range(NCH):
                        nc.vector.tensor_scalar(
                            g1T[:, mc, ts(rb, 128)], ps[:, mc, :],
                            b1T[:, mc : mc + 1], None, op0=ALU.add,
                        )

                def emit_capproj():
                    for half in range(2):
                        ps = p1ps.tile([128, NCH, 128], fp, tag="p1")
                        w = 124
                        for mc in range(NCH):
                            for ko in range(2):
                                nc.tensor.matmul(
                                    ps[:, mc, 0:w], wd2cT[:, ko, mc, :],
                                    capT[:, ko, ds(half * w, w)],
                                    start=(ko == 0), stop=(ko == 1),
                                )
                        for mc in range(NCH):
                            nc.vector.tensor_scalar(
                                capg[:, mc, ds(half * w, w)], ps[:, mc, 0:w],
                                bd2T[:, mc : mc + 1], None, op0=ALU.add,
                            )

                emit_p1_block(0)
                cell(mm1(0), E1t, h1T)
                for t in range(T):
                    ps1n = mm1(t + 1) if t + 1 < T else None
                    ps2 = mm2(t)
                    if ps1n is not None:
                        cell(ps1n, E1t, h1T)
                    cell(ps2, E2t, h2seqT[:, :, t, :])
                    if t in (0, 16, 32, 48):
                        emit_p1_block(t // 16 + 1)
                    elif t == 56:
                        emit_capproj()

            # ================= phase 2: M2R (second half; first half was
            # emitted mid-encoder via emit_m2r) =================
            with ExitStack() as midctx:
                m2ps = midctx.enter_context(
                    tc.tile_pool(name="m2ps", bufs=2, space="PSUM")
                )
                emit_m2r(m2ps, 0, 80, "m2")

            # ================= phase 3: decoder =================
            smallp = ctx.enter_context(
                tc.tile_pool(name="smallp", bufs=1, space="PSUM")
            )
            ceps = ctx.enter_context(
                tc.tile_pool(name="ceps", bufs=2, space="PSUM")
            )
            junkp = ctx.enter_context(tc.tile_pool(name="junk", bufs=2))

            def cell_pool(ps, E, hT_dst):
                nc.scalar.activation(E[:, 0:8, :], ps[:, :, :], AF.Tanh)
                p1 = acts.tile([128, 4, BS], fp, tag="pp1")
                nc.gpsimd.tensor_scalar_add(p1, E[:, 2:6, :], 1.0)
                uv = acts.tile([128, 4, BS], fp, tag="puv")
                nc.gpsimd.tensor_tensor(uv, p1, E[:, 6:10, :], op=ALU.mult)
                uh = acts.tile([128, 2, BS], fp, tag="puh")
                nc.gpsimd.tensor_scalar_mul(uh, uv[:, 2:4, :], 0.5)
                nc.gpsimd.tensor_tensor(
                    E[:, 8:10, :], uh, uv[:, 0:2, :], op=ALU.add
                )
                th = acts.tile([128, 2, BS], fp, tag="pth")
                nc.scalar.activation(th, E[:, 8:10, :], AF.Tanh, scale=0.5)
                so = acts.tile([128, 2, BS], fp, tag="pso")
                nc.gpsimd.tensor_scalar_add(so, E[:, 0:2, :], 1.0)
                nc.gpsimd.tensor_tensor(hT_dst, so, th, op=ALU.mult)

            def d1step(s):
                ps = gps.tile([128, NCH, BS], fp, tag="cell", name="pd1",
                              padded_shape=[128, NCH, 64])
                rhs = h1T if s == 0 else h1decT[:, :, s - 1, :]
                for c in range(NCH):
                    nc.tensor.matmul(
                        ps[:, c, :], bd1row[:, ts(c, 128)], ones1[:, 0:BS],
                        start=True, stop=False,
                    )
                for c in range(NCH):
                    nc.tensor.matmul(
                        ps[:, c, :], wd1T[:, :, c, :], rhs,
                        start=False, stop=True, perf_mode=DR,
                    )
                cell_pool(ps, E1t, h1decT[:, :, s, :])

            def mm2d_pre(t):
                ps = gps.tile([128, NCH, BS], fp, tag="cell", name="pd2",
                              padded_shape=[128, NCH, 64])
                for c in range(NCH):
                    nc.tensor.matmul(
                        ps[:, c, :], identb, capg[:, c, ts(t, BS)],
                        start=True, stop=False,
                    )
                for c in range(NCH):
                    nc.tensor.matmul(
                        ps[:, c, :], wd2iT[:, :, c, :], h1decT[:, :, t, :],
                        start=False, stop=False, perf_mode=DR,
                    )
                return ps

            def attn(t):
                stps = smallp.tile([80, BS], fp, tag="sm",
                                   padded_shape=[80, 512])
                for b in range(BS):
                    nc.tensor.matmul(
                        stps[:, b : b + 1], h2seqT[:, :, :, b],
                        h2decT[:, :, t, b : b + 1],
                        start=True, stop=True, perf_mode=DR,
                    )
                u = acts.tile([80, BS], fp, tag="u")
                nc.scalar.activation(u, stps, AF.Exp, scale=0.25)
                sbc = acts.tile([80, BS], fp, tag="sbc")
                nc.gpsimd.partition_all_reduce(
                    sbc, u, channels=80, reduce_op=bass_isa.ReduceOp.add
                )
                rbc = acts.tile([80, BS], fp, tag="rbc")
                nc.vector.reciprocal(rbc, sbc)
                u_n = acts.tile([80, BS], bf, tag="u_n")
                nc.vector.tensor_tensor(u_n, u, rbc, op=ALU.mult)
                return u_n

            # CE row-groups: g0 = t[0:15] (120 rows), g1 = t[15:31] (128 rows)
            CE_T0 = [0, 15]
            CE_TN = [15, 16]

            def ce_chunk(g, c):
                R = CE_TN[g] * BS
                c0 = 1024 * c
                cw = VCW[c]
                psL = ceps.tile([128, 1024], fp, tag="ce",
                                padded_shape=[128, 1024])
                for j in range(2):
                    w = min(cw - 512 * j, 512)
                    if w <= 0:
                        continue
                    nc.tensor.matmul(
                        psL[:R, ds(512 * j, w)], ones1[:, 0:R],
                        boutrow[:, ds(c0 + 512 * j, w)],
                        start=True, stop=False,
                    )
                    nc.tensor.matmul(
                        psL[:R, ds(512 * j, w)],
                        h2decT[:, :, ds(CE_T0[g], CE_TN[g]), :],
                        wo[:, :, ds(c0 + 512 * j, w)],
                        start=False, stop=True, perf_mode=DR,
                    )
                # exp output is junk (only accum matters): write in-place
                # into the PSUM tile -> cheaper ACT access init than SBUF out
                nc.scalar.activation(
                    psL[:R, 0:cw], psL[:R, 0:cw], AF.Exp,
                    accum_out=s_all[:R, g, c : c + 1],
                )

            for s in range(DLEAD):
                d1step(s)
            pd2 = mm2d_pre(0)
            for c in range(NCH):
                nc.tensor.matmul(
                    pd2[:, c, :], wd2hT[:, :, c, :], h2seqT[:, :, T - 1, :],
                    start=False, stop=True, perf_mode=DR,
                )
            for t in range(DEC):
                cell(pd2, E2t, h2decT[:, :, t, :])
                if t < DEC - 1:
                    u_n = attn(t)
                # CE group-0 chunks sprinkled every other step from t=15
                if 15 <= t < 15 + 2 * NVC and (t - 15) % 2 == 0:
                    ce_chunk(0, (t - 15) // 2)
                if t < DEC - 1:
                    pd2 = mm2d_pre(t + 1)
                    for c in range(NCH):
                        for b in range(BS):
                            nc.tensor.matmul(
                                pd2[:, c, b : b + 1],
                                M2R[:, b, ts(c, 128)], u_n[:, b : b + 1],
                                start=False, stop=(b == BS - 1),
                            )
                if t + DLEAD < DEC:
                    d1step(t + DLEAD)

            # ================= phase 4: CE tail + finals =================
            for c in range(NVC):
                ce_chunk(1, c)

            # tv: -(sum over all rows of h2decT * wotgt)
            jk = junkp.tile([128, 2, DEC, BS], bf, tag="jk2")
            pt = acts.tile([128, 1], fp, tag="pt")
            nc.vector.scalar_tensor_tensor(
                jk, h2decT[:, :, 0:DEC, :], -1.0, wotgt,
                op0=ALU.mult, op1=ALU.mult, accum_out=pt,
            )
            # lse: per-group row sums of exp -> ln -> partition-sum
            fin = smallp.tile([1, 1], fp, tag="sm", padded_shape=[1, 512])
            for g in range(2):
                R = CE_TN[g] * BS
                Sg = acts.tile([128, 1], fp, tag="Sg")
                nc.vector.tensor_reduce(
                    Sg[:R], s_all[:R, g, :], op=ALU.add, axis=AX.X
                )
                lnS = acts.tile([128, 1], fp, tag="lnS")
                nc.scalar.activation(lnS[:R], Sg[:R], AF.Ln)
                nc.tensor.matmul(
                    fin, lnS[:R], onesc[:R], start=(g == 0), stop=False
                )
            nc.tensor.matmul(fin, pt, onesc, start=False, stop=True)
            outsb = acts.tile([1, 1], fp, tag="osb")
            nc.scalar.mul(outsb, fin, 1.0 / (B * B))
            nc.sync.dma_start(out_d[:, :], outsb)

    nc.compile()
    return nc


def _shard_inputs(inputs):
    import ml_dtypes

    f32 = np.float32
    bf16 = ml_dtypes.bfloat16
    f8 = ml_dtypes.float8_e4m3fn
    feat = np.asarray(inputs["feat"], f32)
    caption = np.asarray(inputs["caption"], f32)
    oh = np.asarray(inputs["caption_one_hot"], f32)

    # torch gate order [i f g o] -> chunk order [o i f g]
    perm = np.concatenate(
        [np.arange(3 * H, 4 * H), np.arange(0, H),
         np.arange(H, 2 * H), np.arange(2 * H, 3 * H)]
    )
    rowscale = np.concatenate(
        [np.full(3 * H, 0.5, f32), np.ones(H, f32)]
    )

    def pscale(Wname, cols=None, hpart=False):
        W = np.asarray(inputs[Wname], f32)
        if cols is not None:
            W = W[:, cols]
        Wp = W[perm] * rowscale[:, None]
        if hpart:
            Wp = Wp * 0.5
        return Wp

    def bscale(bname):
        return np.asarray(inputs[bname], f32)[perm] * rowscale

    def statT(Wp, dt):
        # [G, 256] -> [128, 2, 8, 128]
        return np.ascontiguousarray(
            Wp.T.reshape(2, 128, NCH, 128).transpose(1, 0, 2, 3).astype(dt)
        )

    def rhsT(M, dt):
        # [256, N] -> [128, 2, N]
        N = M.shape[1]
        return np.ascontiguousarray(
            M.reshape(2, 128, N).transpose(1, 0, 2).astype(dt)
        )

    W1p = pscale("e1_Wih")                                 # [1024, 4096]
    w1T = np.ascontiguousarray(
        W1p.T.reshape(K2, 2, 128, NCH, 128).transpose(2, 0, 1, 3, 4).astype(f8)
    )
    w1hT = statT(pscale("e1_Whh", hpart=True), f8)
    w2iT = statT(pscale("e2_Wih", cols=slice(H, 2 * H), hpart=True), f8)
    w2hT = statT(pscale("e2_Whh", hpart=True), f8)
    wd1T = statT(pscale("d1_Whh", hpart=True), f8)
    wd2iT = statT(pscale("d2_Wih", cols=slice(H, 2 * H), hpart=True), f8)
    wd2hT = statT(pscale("d2_Whh", hpart=True), f8)
    wd2cT = statT(pscale("d2_Wih", cols=slice(0, H)), bf16)
    wh2rhs = rhsT(pscale("d2_Whh", hpart=True).T, f8)      # [256,1024]->...
    Wop = np.asarray(inputs["out_W"], f32) * 0.5           # [8000, 256]
    woT = rhsT(Wop.T, f8)
    out_b = np.asarray(inputs["out_b"], f32)

    b1T = np.ascontiguousarray(bscale("e1_b").reshape(NCH, 128).T.astype(f32))
    bd2T = np.ascontiguousarray(bscale("d2_b").reshape(NCH, 128).T.astype(f32))
    brows = np.concatenate(
        [bscale("e2_b"), bscale("d1_b"), out_b]
    ).reshape(1, 2 * G + V).astype(bf16)

    targets = np.argmax(oh, axis=2)                        # [64, 32]
    btgt_total = float(out_b[targets[:, 1:]].sum())

    shared = dict(
        w1T=w1T, w1hT=w1hT, w2iT=w2iT, w2hT=w2hT, wd1T=wd1T,
        wd2iT=wd2iT, wd2hT=wd2hT, wd2cT=wd2cT, wh2rhs=wh2rhs,
        woT=woT, b1T=b1T, bd2T=bd2T, brows=brows,
    )

    in_maps = []
    for cidx in range(NCORES):
        b0 = cidx * BS
        featT = np.ascontiguousarray(
            feat[b0 : b0 + BS].transpose(2, 1, 0)          # [4096, 80, 8]
            .reshape(FEAT, TB)
            .reshape(K2, 2, 128, NRB, 128)
            .transpose(2, 3, 0, 1, 4)                      # [128, 5, 16, 2, 128]
            .astype(f8)
        )
        capT = np.ascontiguousarray(
            caption[b0 : b0 + BS, :DEC].transpose(2, 1, 0) # [256, 31, 8]
            .reshape(H, CROWS)
            .reshape(2, 128, CROWS).transpose(1, 0, 2)
            .astype(bf16)
        )
        tg = targets[b0 : b0 + BS, 1:]                     # [8, 31]
        sel = Wop[tg.T.reshape(-1)]                        # [248, 256] r=s*8+b
        wotgt = np.ascontiguousarray(
            sel.T.reshape(2, 128, CROWS).transpose(1, 0, 2)
            .reshape(128, 2, DEC, BS).astype(f8)
        )
        m = dict(shared)
        m.update(featT=featT, capT=capT, wotgt=wotgt)
        in_maps.append(m)
    return in_maps, btgt_total


def kernel(**inputs):
    from concourse.bass_utils import run_bass_kernel_spmd

    if "nc" not in _cache:
        _cache["nc"] = _build_program()
    nc = _cache["nc"]
    in_maps, btgt_total = _shard_inputs(inputs)
    res = run_bass_kernel_spmd(nc, in_maps, core_ids=list(range(NCORES)))
    total = 0.0
    for r in res.results:
        total += float(np.asarray(r["partial"]).reshape(-1)[0])
    total -= btgt_total / (B * B)
    return np.asarray(np.float32(total))
